# revision 70
# baseline (speedup 1.0000x reference)
"""Braid causal self-attention Trainium2 kernel (8-core SPMD), v2.

Sharding: data-parallel over batch (2) x tensor-parallel over head groups (4).
Core c handles batch b=c//4, q-heads [4g:4g+4], kv-heads [2g:2g+2], g=c%4.
Each core emits a bf16 partial projection output (Wproj input-dim shard);
partials are summed on the host.

v2 structure (vs v1): bf16 matmul operands everywhere (same PE rate as f32r
but half the LDWEIGHTS/SBUF/DMA cost and no 512-wide PSUM-window constraint),
exact-causal key-block strips, and the per-strip sigmoid split across TWO
engine paths to break the scalar-engine wall (~29% of columns on the
vector-engine path, balancing 1-pass sigmoid vs 2-pass den+recip):
  - ACT strips: attn = Sigmoid(s_q[i] + s_k[j]) on the scalar engine
    (s_k as per-partition activation bias), triangular diag mask on Pool.
  - DVE strips: attn = 1/(e^-s_q[i] * e^-s_k[j] + 1); the rank-1
    denominator is a DVE scalar_tensor_tensor (causal mask folded in via a
    [1e30-above-diag | ones] in1 that also supplies the +1), then a
    reciprocal_approx_fast custom-DVE op straight into f32r for the matmul.
    Scores are clamped to +-30 so the exponentials stay finite (sigmoid is
    saturated there anyway).
Per-token score rows are broadcast to all 128 partitions by DRAM-bounce
broadcast DMAs (no PE/copy cost). Attention y accumulates in a single
[128, T] PSUM region, consecutive heads using opposite partition halves
(matmul tile_position), so two heads overlap without double-banking.
"""
import numpy as np
import ml_dtypes
from contextlib import ExitStack

import concourse.bass as bass
import concourse.mybir as mybir
import concourse.tile as tile
from concourse import bacc
from concourse.bass_utils import run_bass_kernel_spmd
from concourse.dve_ops import RECIP_APPROX_FAST_CONSTS, RECIPROCAL_APPROX_FAST

F32 = mybir.dt.float32
F32R = mybir.dt.float32r
BF16 = mybir.dt.bfloat16
AF = mybir.ActivationFunctionType
ALU = mybir.AluOpType

T = 2048
C = 1024
D = 64
EPS = 1e-6
NCORES = 8
CLAMP = 30.0

# score-row layout: 0=sk_kh0, 1=sk_kh1, 2=sq_h0, 3=sq_h1, 4=sq_h2, 5=sq_h3
def dve_strip(jb):
    return jb % 3 == 2


def build_program():
    nc = bacc.Bacc()
    dp = nc.declare_dram_parameter
    xT_d = dp("xT", [C, T], BF16, isOutput=False)         # x[b].T bf16
    wqkv_d = dp("wqkv", [C, 512], BF16, isOutput=False)   # [Wq.T|Wk.T|Wv.T]
    wp_d = dp("wp", [256, C], BF16, isOutput=False)       # Wproj[:, grp].T (prescaled)
    gm_d = dp("gm", [128, T], BF16, isOutput=False)       # braid g (2-head dup)
    mh_d = dp("mh", [128, T], BF16, isOutput=False)       # sqrt(cos^2+sin^2)
    sel_d = dp("sel", [128, 3, 6], BF16, isOutput=False)  # head selector masks
    tri_d = dp("tri", [128, 128], BF16, isOutput=False)   # tri[p,c]=1 if p<=c
    mko_d = dp("mko", [128, T], F32, isOutput=False)      # [1e30 above diag|ones]
    idn_d = dp("idn", [128, 128], F32, isOutput=False)
    out_d = dp("outp", [T, C], BF16, isOutput=True)

    with tile.TileContext(nc) as tc, \
         nc.allow_low_precision("bf16 matmul operands validated vs fp64 reference"), \
         ExitStack() as ctx:
        cons = ctx.enter_context(tc.tile_pool(name="cons", bufs=1))
        work = ctx.enter_context(tc.tile_pool(name="work", bufs=1))

        # ---- constants / weights in SBUF ----
        wqkv_s = cons.tile([128, 8, 512], BF16)
        wp_s = cons.tile([128, 2, C], BF16)
        sel_s = cons.tile([128, 3, 6], BF16)
        tri_s = cons.tile([128, 128], BF16)
        mko_s = cons.tile([128, T], F32)
        idn_s = cons.tile([128, 128], F32)
        eps_t = cons.tile([128, 1], F32)
        # critical-path inputs (wqkv + first x chunk) go first on qSP, each
        # split by K-half so the first matmuls start ~4us in;
        # later-needed constants ride the scalar engine's DMA queue
        wqr = wqkv_d.ap().rearrange("(kt p) m -> p kt m", p=128)
        nc.sync.dma_start(out=wqkv_s[:, 0:4, :], in_=wqr[:, 0:4, :])
        nc.sync.dma_start(out=wqkv_s[:, 4:8, :], in_=wqr[:, 4:8, :])
        nc.scalar.dma_start(out=sel_s[:], in_=sel_d.ap())
        nc.scalar.dma_start(out=wp_s[:], in_=wp_d.ap().rearrange("(kt p) m -> p kt m", p=128))
        nc.scalar.dma_start(out=tri_s[:], in_=tri_d.ap())
        nc.scalar.dma_start(out=mko_s[:], in_=mko_d.ap())
        nc.scalar.dma_start(out=idn_s[:], in_=idn_d.ap())
        nc.vector.memset(eps_t[:], EPS)

        # long-lived work tiles
        v_bf = work.tile([128, T], BF16)      # 16 blocks of [t128, kh0 d64|kh1 d64]
        kcolT = work.tile([128, 4, 16], F32)  # cols: [sk0, sk1, e^-sk0, e^-sk1]
        hb = [work.tile([128, T], BF16, name=f"hb{h}") for h in range(4)]  # s_q bcast
        ha = [work.tile([128, T], BF16, name=f"ha{h}") for h in range(4)]  # e^-s_q bcast
        yt0 = work.tile([128, T], BF16)       # heads 0,1 y (d-major)
        yt1 = work.tile([128, T], BF16)

        # ==== phases 1+2 (scoped SBUF, released before attention) ====
        ph12_ctx = ExitStack()
        ph12 = ph12_ctx.enter_context(tc.tile_pool(name="ph12", bufs=1))
        vT = ph12.tile([128, T], F32, name="vT")
        stil = ph12.tile([6, T], F32, name="stil")
        r1 = ph12.tile([6, T], F32, name="r1")
        rq = ph12.tile([6, T], F32, name="rq")
        scomp = ph12.tile([6, T], F32, name="scomp")
        erows = ph12.tile([6, T], F32, name="erows")
        prow6 = ph12.tile([6, T], BF16, name="prow6")
        erowb = ph12.tile([6, T], BF16, name="erowb")

        # ==== phase 1: projections with fused braid reductions ====
        with tc.tile_pool(name="bpool", bufs=3) as bp, \
             tc.tile_pool(name="xpool", bufs=4) as xp, \
             tc.tile_pool(name="pp1", bufs=2, space="PSUM") as pp1, \
             tc.tile_pool(name="pp2", bufs=2, space="PSUM") as pp2:
            gm_s = bp.tile([128, T], BF16, tag="gm")
            mh_s = bp.tile([128, T], BF16, tag="mh")
            xr = xT_d.ap().rearrange("(kt p) t -> p kt t", p=128)
            xcs = []
            for cn in range(4):
                xc = xp.tile([128, 8, 512], BF16, tag="x")
                if cn == 0:
                    sl = slice(0, 512)
                    nc.sync.dma_start(out=xc[:, 0:4, :], in_=xr[:, 0:4, sl])
                    nc.sync.dma_start(out=xc[:, 4:8, :], in_=xr[:, 4:8, sl])
                else:
                    nc.sync.dma_start(out=xc[:], in_=xr[:, :, 512 * cn:512 * cn + 512])
                xcs.append(xc)
                if cn == 0:
                    nc.sync.dma_start(out=gm_s[:], in_=gm_d.ap())
                    nc.sync.dma_start(out=mh_s[:], in_=mh_d.ap())

            for cn in range(4):
                sl = slice(512 * cn, 512 * cn + 512)
                pss_t = pp2.tile([6, 512], F32, tag="pss")
                psq_t = pp2.tile([6, 512], F32, tag="psq")
                # the sel matmuls are deferred one row-tile so the in-order
                # PE never stalls waiting for the DVE braid products
                pend = None

                def emit_sel(p):
                    a_t, b2_t, ti = p
                    nc.tensor.matmul(pss_t[:], sel_s[:, ti, :], a_t[:],
                                     start=(ti == 0), stop=(ti == 2))
                    nc.tensor.matmul(psq_t[:], sel_s[:, ti, :], b2_t[:],
                                     start=(ti == 0), stop=(ti == 2))

                for t_i, oc0 in ((0, 0), (1, 128), (2, 256), (3, 384)):
                    ps = pp1.tile([128, 512], F32, tag="pj")
                    for kt in range(8):
                        nc.tensor.matmul(
                            ps[:], wqkv_s[:, kt, oc0:oc0 + 128],
                            xcs[cn][:, kt, :],
                            start=(kt == 0), stop=(kt == 7))
                    if pend is not None:
                        emit_sel(pend)
                        pend = None
                    if t_i == 3:
                        nc.vector.tensor_copy(vT[:, sl], ps[:])
                    else:
                        a_t = bp.tile([128, 512], BF16, tag="a")
                        b_t = bp.tile([128, 512], BF16, tag="b")
                        b2_t = bp.tile([128, 512], BF16, tag="b2")
                        nc.vector.tensor_mul(a_t[:], ps[:], gm_s[:, sl])
                        nc.vector.tensor_mul(b_t[:], ps[:], mh_s[:, sl])
                        nc.scalar.activation(b2_t[:], b_t[:], AF.Square)
                        pend = (a_t, b2_t, t_i)
                if pend is not None:
                    emit_sel(pend)
                nc.scalar.copy(stil[:, sl], pss_t[:])
                nc.scalar.activation(r1[:, sl], psq_t[:], AF.Ln,
                                     bias=eps_t[0:6], scale=1.0 / 64.0)

            # v transposes: 16 x [128,128] -> [key128, d] blocks; bf16+f32r copies
            for grp in range(4):
                ps_t = pp1.tile([128, 512], F32, tag="pj")
                for k in range(4):
                    jb = 4 * grp + k
                    nc.tensor.transpose(
                        ps_t[:, 128 * k:128 * k + 128],
                        vT[:, 128 * jb:128 * jb + 128], idn_s[:])
                sl = slice(512 * grp, 512 * grp + 512)
                nc.scalar.copy(v_bf[:, sl], ps_t[:])

        # ==== phase 2: score rows, clamp, exponentials, bounces ====
        nc.scalar.activation(rq[:], r1[:], AF.Exp, scale=-0.5)
        nc.vector.tensor_mul(r1[:], stil[:], rq[:])
        # clamp +-30 (saturated sigmoid; keeps e^-s * e^-s finite)
        nc.vector.tensor_scalar(scomp[:], r1[:], -CLAMP, CLAMP, ALU.max, ALU.min)
        # compute-ops must start at partition 0/32/64/96 -> do all 6 rows
        nc.scalar.activation(erows[:], scomp[:], AF.Exp, scale=-1.0)
        nc.vector.tensor_copy(prow6[:], scomp[:])
        nc.vector.tensor_copy(erowb[:], erows[:])
        # preload the sigmoid table while the broadcast DMAs run
        nc.scalar.activation(rq[0:6, 0:1], erows[0:6, 0:1], AF.Sigmoid)

        # column bounces: s_k / e^-s_k as [128, 1] columns per key block
        ksc_d = nc.dram_tensor("kscratch", [4, T], F32)
        nc.sync.dma_start(out=ksc_d.ap()[0:2, :], in_=scomp[0:2, :])
        nc.sync.dma_start(out=ksc_d.ap()[2:4, :], in_=erows[0:2, :])
        nc.sync.dma_start(out=kcolT[:], in_=ksc_d.ap().rearrange("r (b j) -> j r b", j=128))

        # row broadcasts: s_q / e^-s_q to all 128 partitions via DRAM bounce
        # (sync HWDGE + gpsimd SWDGE queues; keep the scalar engine free, and
        # land hb[0]/hb[1] first so head-0 sigmoids start ASAP)
        brow_d = nc.dram_tensor("browscratch", [8, T], BF16)
        nc.sync.dma_start(out=brow_d.ap()[0:4, :], in_=prow6[2:6, :])
        nc.gpsimd.dma_start(out=brow_d.ap()[4:8, :], in_=erowb[2:6, :])

        def bcast(eng, dst, row):
            eng.dma_start(out=dst[:], in_=brow_d.ap()[row:row + 1, :].to_broadcast((128, T)))

        bcast(nc.sync, hb[0], 0)
        bcast(nc.gpsimd, ha[0], 4)
        bcast(nc.sync, hb[1], 1)
        bcast(nc.gpsimd, ha[1], 5)
        bcast(nc.sync, hb[2], 2)
        bcast(nc.gpsimd, ha[2], 6)
        bcast(nc.sync, hb[3], 3)
        bcast(nc.gpsimd, ha[3], 7)
        ph12_ctx.close()

        # ==== phase 3: attention ====
        with tc.tile_pool(name="atp", bufs=8) as atp, \
             tc.tile_pool(name="dnp", bufs=3) as dnp, \
             tc.tile_pool(name="afp", bufs=4) as afp, \
             tc.tile_pool(name="yp", bufs=1, space="PSUM") as yp:
            y_ps = yp.tile([128, T], F32)

            def strip(h, jb):
                kh = h // 2
                r0 = 64 * (h % 2)
                if True:
                    j0 = 128 * jb
                    W = T - j0
                    if not dve_strip(jb):
                        at = atp.tile([128, T], BF16, tag="at")
                        nc.scalar.activation(at[:, 0:W], hb[h][:, j0:T],
                                             AF.Sigmoid,
                                             bias=kcolT[:, kh, jb:jb + 1])
                        nc.vector.tensor_mul(at[:, 0:128], at[:, 0:128], tri_s[:])
                        rhs_t = at
                    else:
                        den = dnp.tile([128, T], F32, tag="den")
                        nc.vector.scalar_tensor_tensor(
                            den[:, 0:W], ha[h][:, j0:T],
                            kcolT[:, 2 + kh, jb:jb + 1], mko_s[:, 0:W],
                            ALU.mult, ALU.add)
                        # reciprocal_approx_fast body with bf16 out (the
                        # fp32-bit-layout trick constrains the INPUT only)
                        at32 = afp.tile([128, T], BF16, tag="a32")
                        rc = RECIP_APPROX_FAST_CONSTS
                        nc.vector._custom_dve(
                            RECIPROCAL_APPROX_FAST, out=at32[:, 0:W],
                            in0=den[:, 0:W], s0=rc["s0"], s1=rc["s1"],
                            imm2=rc["imm2"])
                        rhs_t = at32
                    # exact-causal chunks, split at 512 (PSUM bank) boundaries
                    c0 = j0
                    while c0 < T:
                        c1 = min((c0 // 512 + 1) * 512, T)
                        bank = c0 // 512
                        nc.tensor.matmul(
                            y_ps[r0:r0 + 64, c0:c1],
                            v_bf[:, 128 * jb + 64 * kh:128 * jb + 64 * kh + 64],
                            rhs_t[:, c0 - j0:c1 - j0],
                            start=(jb == 0), stop=(jb == min(15, 4 * bank + 3)))
                        c0 = c1

            # sequential heads; each head's yt copy (DVE) is deferred into
            # the NEXT head's stream so neither ACT nor DVE hard-syncs on a
            # head boundary
            pending = None
            for h in range(4):
                for jb in range(16):
                    strip(h, jb)
                    if jb == 1 and pending is not None:
                        ph = pending
                        nc.vector.tensor_copy(
                            (yt0 if ph < 2 else yt1)[64 * (ph % 2):64 * (ph % 2) + 64, :],
                            y_ps[64 * (ph % 2):64 * (ph % 2) + 64, :])
                        pending = None
                pending = h
            nc.vector.tensor_copy(yt1[64:128, :], y_ps[64:128, :])

        # ==== phase 4: output projection (both K halves, single write) ====
        with tc.tile_pool(name="ostage", bufs=3) as osp, \
             tc.tile_pool(name="pp4", bufs=3, space="PSUM") as pp4:
            for tt in range(16):
                ps_o = pp4.tile([128, 1024], F32, tag="opj")
                for cn2 in (0, 512):
                    for k2, yt_src in enumerate((yt0, yt1)):
                        nc.tensor.matmul(
                            ps_o[:, cn2:cn2 + 512],
                            yt_src[:, 128 * tt:128 * tt + 128],
                            wp_s[:, k2, cn2:cn2 + 512],
                            start=(k2 == 0), stop=(k2 == 1))
                o_t = osp.tile([128, 1024], BF16, tag="ost")
                if tt % 2:
                    nc.scalar.copy(o_t[:], ps_o[:])
                else:
                    nc.vector.tensor_copy(o_t[:], ps_o[:])
                nc.sync.dma_start(
                    out=out_d.ap()[128 * tt:128 * tt + 128, :], in_=o_t[:])

    nc.compile()
    return nc


_PROGRAM = None


def _get_program():
    global _PROGRAM
    if _PROGRAM is None:
        _PROGRAM = build_program()
    return _PROGRAM


def _host_inputs(x, cos, sin, Wq, Wk, Wv, Wproj, w_braid):
    bf = ml_dtypes.bfloat16
    cos2 = cos[:, 0, :].astype(np.float32)   # [T, 32]
    sin2 = sin[:, 0, :].astype(np.float32)
    wb = w_braid.astype(np.float32)
    g64 = np.empty((64, T), np.float32)
    g64[:32] = wb[:32, None] * cos2.T - wb[32:, None] * sin2.T
    g64[32:] = wb[32:, None] * cos2.T + wb[:32, None] * sin2.T
    gm = np.concatenate([g64, g64], axis=0).astype(bf)
    mh1 = np.sqrt(cos2.T ** 2 + sin2.T ** 2).astype(np.float32)  # [32, T]
    mh64 = np.concatenate([mh1, mh1], axis=0)
    mh = np.concatenate([mh64, mh64], axis=0).astype(bf)

    # score-row layout: 0=sk_kh0, 1=sk_kh1, 2=sq_h0, 3=sq_h1, 4=sq_h2, 5=sq_h3
    sel = np.zeros((128, 3, 6), np.float32)
    sel[0:64, 0, 2] = 1.0    # q tile 0: head 0 -> row 2
    sel[64:128, 0, 3] = 1.0  # head 1 -> row 3
    sel[0:64, 1, 4] = 1.0    # q tile 1: head 2 -> row 4
    sel[64:128, 1, 5] = 1.0  # head 3 -> row 5
    sel[0:64, 2, 0] = 1.0    # k tile: kh0 -> row 0
    sel[64:128, 2, 1] = 1.0  # kh1 -> row 1
    sel = sel.astype(bf)

    tri = (np.arange(128)[:, None] <= np.arange(128)[None, :]).astype(bf)
    mko = np.ones((128, T), np.float32)
    mko[:, 0:128] = np.where(
        np.arange(128)[:, None] > np.arange(128)[None, :], 1e30, 1.0)
    idn = np.eye(128, dtype=np.float32)
    pscale = np.float32(1.0 / (T ** 0.5 + 1e-6))

    in_maps = []
    for c in range(NCORES):
        b, g = c // 4, c % 4
        wqkv = np.concatenate([
            Wq[256 * g:256 * (g + 1)].T,
            Wk[128 * g:128 * (g + 1)].T,
            Wv[128 * g:128 * (g + 1)].T], axis=1)
        in_maps.append({
            "xT": np.ascontiguousarray(x[b].T).astype(bf),
            "wqkv": np.ascontiguousarray(wqkv).astype(bf),
            "wp": np.ascontiguousarray(
                (Wproj[:, 256 * g:256 * (g + 1)] * pscale).T).astype(bf),
            "gm": gm, "mh": mh, "sel": sel, "tri": tri, "mko": mko,
            "idn": idn,
        })
    return in_maps


def kernel(x, cos, sin, Wq, Wk, Wv, Wproj, w_braid):
    nc = _get_program()
    in_maps = _host_inputs(np.asarray(x, np.float32), np.asarray(cos), np.asarray(sin),
                           np.asarray(Wq, np.float32), np.asarray(Wk, np.float32),
                           np.asarray(Wv, np.float32), np.asarray(Wproj, np.float32),
                           np.asarray(w_braid, np.float32))
    res = run_bass_kernel_spmd(nc, in_maps, list(range(NCORES)))
    out = np.zeros((2, T, C), np.float32)
    for c in range(NCORES):
        out[c // 4] += res.results[c]["outp"].astype(np.float32)
    return out
